# revision 1
# baseline (speedup 1.0000x reference)
"""PatchCore anomaly score kernel for 8 trn2 NeuronCores.

score = sqrt(max_n min_m ||patches[n] - memory_bank[m]||^2)

Device (per core, memory_bank sharded 4096 rows/core):
  r_c[n] = max_m (2*patches[n].bank[m] - (m_sq[m] - C))     [MAX-only ops]
Host:
  min_d2[n] = p_sq[n] + C - max_c r_c[n];  score = sqrt(max_n min_d2)
"""

import sys

import numpy as np

try:
    import concourse.bass as bass
except ImportError:
    sys.path.insert(0, "/opt/trn_rl_repo")
    import concourse.bass as bass

import concourse.bacc as bacc
import concourse.tile as tile
from concourse import mybir
from concourse.bass_utils import run_bass_kernel_spmd

import ml_dtypes

N = 8192          # patches
M_TOTAL = 32768   # memory bank rows
D = 512           # feature dim
N_CORES = 8
M = M_TOTAL // N_CORES   # 4096 bank rows per core

KP = 4            # k-chunks of 128 over D
NT = N // 512     # 16 n-tiles of 512 patches
MT = M // 128     # 32 m-tiles of 128 bank rows

# Measured variants (For_i-amplified bench, ns/iter): bf16 432k;
# fp8 DoubleRow + ACT-bias evac + DVE max (this config, STT=False) 149k;
# STT=True (odd m-tiles via one DVE scalar_tensor_tensor from PSUM) 235k
# -- the serial DVE RMW chain on rmax with 512-cycle f32 PSUM reads
# paces worse than the split ACT+DVE pipeline, so it stays off.
STT = False


def _build_nc(repeat=1, fp8=True):
    # Bacc (not Bass): its compile() pass splits multi-sem waits into
    # event semaphores — TRN2 allows only 1 embedded wait per instruction.
    # repeat>1 wraps the compute in a For_i hardware loop (bench-only:
    # amplifies device work so wall-clock deltas resolve the kernel time).
    if fp8:
        return _build_nc_fp8(repeat)
    nc = bacc.Bacc(None, target_bir_lowering=False)
    f32 = mybir.dt.float32
    bf16 = mybir.dt.bfloat16

    at_d = nc.dram_tensor("at", [D, N], bf16, kind="ExternalInput")
    bt_d = nc.dram_tensor("bt", [D, M], bf16, kind="ExternalInput")
    msq_d = nc.dram_tensor("msq", [128, MT], f32, kind="ExternalInput")
    id_d = nc.dram_tensor("ident", [128, 128], bf16, kind="ExternalInput")
    # out[p, blk] = r[blk*128 + p]; host transposes
    out_d = nc.dram_tensor("out", [128, NT * 4], f32, kind="ExternalOutput")

    with tile.TileContext(nc) as tc:
        with (
            tc.tile_pool(name="at", bufs=1) as at_pool,
            tc.tile_pool(name="bt", bufs=1) as bt_pool,
            tc.tile_pool(name="msq", bufs=1) as msq_pool,
            tc.tile_pool(name="rmax", bufs=2) as rmax_pool,
            tc.tile_pool(name="evac", bufs=4) as evac_pool,
            tc.tile_pool(name="res", bufs=1) as res_pool,
            tc.tile_pool(name="psum", bufs=6, space="PSUM") as psum_pool,
            tc.tile_pool(name="psumt", bufs=2, space="PSUM") as psumt_pool,
        ):
            msq_t = msq_pool.tile([128, MT], f32)
            nc.gpsimd.dma_start(msq_t[:], msq_d[:])
            id_t = msq_pool.tile([128, 128], bf16, name="id_t")
            nc.gpsimd.dma_start(id_t[:], id_d[:])
            res_t = res_pool.tile([128, NT * 4], f32)

            # bank first (whole bank needed for n-tile 0), in k/col chunks
            bt_t = [bt_pool.tile([128, M], bf16, name=f"bt{k}") for k in range(KP)]
            for k in range(KP):
                for j in range(4):
                    nc.gpsimd.dma_start(
                        bt_t[k][:, bass.ts(j, M // 4)],
                        bt_d[bass.ts(k, 128), bass.ts(j, M // 4)],
                    )
            # patches, in column chunks matching n-tile consumption order
            at_t = [at_pool.tile([128, N], bf16, name=f"at{k}") for k in range(KP)]
            for j in range(8):
                for k in range(KP):
                    nc.gpsimd.dma_start(
                        at_t[k][:, bass.ts(j, N // 8)],
                        at_d[bass.ts(k, 128), bass.ts(j, N // 8)],
                    )

            def reduce_ntile(n, rmax):
                # partition-axis max of rmax [128,512] via PE transpose
                # of each 128-col chunk + DVE free-axis max reduce.
                for c in range(4):
                    pst = psumt_pool.tile([128, 128], bf16, name="pst")
                    nc.tensor.transpose(
                        pst[:], rmax[:, bass.ts(c, 128)], id_t[:]
                    )
                    col = n * 4 + c
                    nc.vector.tensor_reduce(
                        res_t[:, col : col + 1], pst[:],
                        mybir.AxisListType.X, mybir.AluOpType.max,
                    )

            def compute_body():
                prev = None
                for n in range(NT):
                    rmax = rmax_pool.tile([128, 512], bf16)
                    for m in range(MT):
                        ps = psum_pool.tile([128, 512], f32)
                        for k in range(KP):
                            nc.tensor.matmul(
                                ps[:],
                                bt_t[k][:, bass.ts(m, 128)],
                                at_t[k][:, bass.ts(n, 512)],
                                start=(k == 0),
                                stop=(k == KP - 1),
                            )
                        if m == 0:
                            nc.scalar.activation(
                                rmax[:], ps[:],
                                mybir.ActivationFunctionType.Identity,
                                bias=msq_t[:, m : m + 1], scale=2.0,
                            )
                        else:
                            ev = evac_pool.tile([128, 512], bf16)
                            nc.scalar.activation(
                                ev[:], ps[:],
                                mybir.ActivationFunctionType.Identity,
                                bias=msq_t[:, m : m + 1], scale=2.0,
                            )
                            nc.vector.tensor_max(rmax[:], rmax[:], ev[:])
                        if m == 4 and prev is not None:
                            reduce_ntile(n - 1, prev)  # keep PE busy before stall
                    prev = rmax
                reduce_ntile(NT - 1, prev)

            if repeat == 1:
                compute_body()
            else:
                with tc.For_i(0, repeat):
                    compute_body()
            nc.gpsimd.dma_start(out_d[:], res_t[:])

    nc.finalize()
    return nc


def _build_nc_fp8(repeat=1):
    """fp8e4m3 DoubleRow variant: K=256 per matmul, 2x PE throughput.

    Device result is approximate; kernel() refines the top candidates
    exactly on host (inputs are small enough that a few exact rows of
    d2 cost ~0.1s in BLAS).
    """
    nc = bacc.Bacc(None, target_bir_lowering=False)
    f32 = mybir.dt.float32
    bf16 = mybir.dt.bfloat16
    fp8 = mybir.dt.float8e4

    # dim1 index ci = d//128; slice [:, 2c:2c+2, :] = K rows [c*256,(c+1)*256)
    at_d = nc.dram_tensor("at", [128, KP, N], fp8, kind="ExternalInput")
    bt_d = nc.dram_tensor("bt", [128, KP, M], fp8, kind="ExternalInput")
    msq_d = nc.dram_tensor("msq", [128, MT], f32, kind="ExternalInput")
    id_d = nc.dram_tensor("ident", [128, 128], bf16, kind="ExternalInput")
    out_d = nc.dram_tensor("out", [128, NT * 4], f32, kind="ExternalOutput")

    with tile.TileContext(nc) as tc:
        with (
            tc.tile_pool(name="at", bufs=1) as at_pool,
            tc.tile_pool(name="bt", bufs=1) as bt_pool,
            tc.tile_pool(name="msq", bufs=1) as msq_pool,
            tc.tile_pool(name="rmax", bufs=2) as rmax_pool,
            tc.tile_pool(name="evac", bufs=4) as evac_pool,
            tc.tile_pool(name="res", bufs=1) as res_pool,
            tc.tile_pool(name="psum", bufs=6, space="PSUM") as psum_pool,
            tc.tile_pool(name="psumt", bufs=2, space="PSUM") as psumt_pool,
        ):
            msq_t = msq_pool.tile([128, MT], f32)
            nc.gpsimd.dma_start(msq_t[:], msq_d[:])
            id_t = msq_pool.tile([128, 128], bf16, name="id_t")
            nc.gpsimd.dma_start(id_t[:], id_d[:])
            res_t = res_pool.tile([128, NT * 4], f32)

            bt_t = bt_pool.tile([128, KP, M], fp8)
            for ci in range(KP):
                for j in range(4):
                    nc.gpsimd.dma_start(
                        bt_t[:, ci, bass.ts(j, M // 4)],
                        bt_d[:, ci, bass.ts(j, M // 4)],
                    )
            at_t = at_pool.tile([128, KP, N], fp8)
            for j in range(8):
                for ci in range(KP):
                    nc.gpsimd.dma_start(
                        at_t[:, ci, bass.ts(j, N // 8)],
                        at_d[:, ci, bass.ts(j, N // 8)],
                    )

            def reduce_ntile(n, rmax):
                for c in range(4):
                    pst = psumt_pool.tile([128, 128], bf16, name="pst")
                    nc.tensor.transpose(
                        pst[:], rmax[:, bass.ts(c, 128)], id_t[:]
                    )
                    col = n * 4 + c
                    nc.vector.tensor_reduce(
                        res_t[:, col : col + 1], pst[:],
                        mybir.AxisListType.X, mybir.AluOpType.max,
                    )

            act_scale = 1.0 if STT else 2.0

            def compute_body():
                prev = None
                for n in range(NT):
                    rmax = rmax_pool.tile([128, 512], bf16)
                    for m in range(MT):
                        ps = psum_pool.tile([128, 512], f32)
                        for c in range(2):
                            nc.tensor.matmul(
                                ps[:],
                                bt_t[:, 2 * c : 2 * c + 2, bass.ts(m, 128)],
                                at_t[:, 2 * c : 2 * c + 2, bass.ts(n, 512)],
                                start=(c == 0),
                                stop=(c == 1),
                                perf_mode=mybir.MatmulPerfMode.DoubleRow,
                            )
                        if m == 0:
                            nc.scalar.activation(
                                rmax[:], ps[:],
                                mybir.ActivationFunctionType.Identity,
                                bias=msq_t[:, m : m + 1], scale=act_scale,
                            )
                        elif STT and m % 2 == 1:
                            nc.vector.scalar_tensor_tensor(
                                rmax[:], ps[:], msq_t[:, m : m + 1], rmax[:],
                                mybir.AluOpType.add, mybir.AluOpType.max,
                            )
                        else:
                            ev = evac_pool.tile([128, 512], bf16)
                            nc.scalar.activation(
                                ev[:], ps[:],
                                mybir.ActivationFunctionType.Identity,
                                bias=msq_t[:, m : m + 1], scale=act_scale,
                            )
                            nc.vector.tensor_max(rmax[:], rmax[:], ev[:])
                        if m == 4 and prev is not None:
                            reduce_ntile(n - 1, prev)
                    prev = rmax
                reduce_ntile(NT - 1, prev)

            if repeat == 1:
                compute_body()
            else:
                with tc.For_i(0, repeat):
                    compute_body()
            nc.gpsimd.dma_start(out_d[:], res_t[:])

    nc.finalize()
    return nc


_NC = None


def prepare_in_maps(patches: np.ndarray, memory_bank: np.ndarray, fp8=True):
    m_sq = np.sum(
        memory_bank.astype(np.float64) ** 2, axis=1
    )
    C = float(np.mean(m_sq))
    id_np = np.eye(128, dtype=ml_dtypes.bfloat16)
    if fp8:
        # [128, 4, N] with dim1 = d//128 (K-chunk index).
        # STT folds the x2 of the distance expansion into the patches
        # (power-of-2 scale: exact in fp8, no extra quantization error).
        pt = patches.T * 2.0 if STT else patches.T
        at_np = np.ascontiguousarray(
            pt.astype(ml_dtypes.float8_e4m3)
            .reshape(KP, 128, N).transpose(1, 0, 2)
        )
    else:
        at_np = np.ascontiguousarray(patches.T).astype(ml_dtypes.bfloat16)
    in_maps = []
    for c in range(N_CORES):
        bank_c = memory_bank[c * M : (c + 1) * M]
        if fp8:
            bt_np = np.ascontiguousarray(
                bank_c.T.astype(ml_dtypes.float8_e4m3)
                .reshape(KP, 128, M).transpose(1, 0, 2)
            )
        else:
            bt_np = np.ascontiguousarray(bank_c.T).astype(ml_dtypes.bfloat16)
        msq_c = -(m_sq[c * M : (c + 1) * M] - C)
        msq_np = np.ascontiguousarray(
            msq_c.reshape(MT, 128).T
        ).astype(np.float32)
        in_maps.append({"at": at_np, "bt": bt_np, "msq": msq_np, "ident": id_np})
    return in_maps


def kernel(patches: np.ndarray, memory_bank: np.ndarray) -> np.ndarray:
    global _NC
    if _NC is None:
        _NC = _build_nc()
    nc = _NC

    p64 = patches.astype(np.float64)
    b64 = memory_bank.astype(np.float64)
    p_sq = np.sum(p64 * p64, axis=1)          # [N]
    m_sq = np.sum(b64 * b64, axis=1)          # [M_TOTAL]
    C = float(np.mean(m_sq))

    in_maps = prepare_in_maps(patches, memory_bank)

    br = run_bass_kernel_spmd(nc, in_maps, list(range(N_CORES)))
    r = np.max(
        np.stack(
            [np.asarray(br.results[c]["out"], np.float64).T.reshape(N)
             for c in range(N_CORES)]
        ),
        axis=0,
    )
    min_d2 = np.maximum(p_sq + C - r, 0.0)

    # Host refinement: device min_d2 is approximate (fp8 matmul + bf16 max
    # accumulation). Recompute exact d2 rows for every candidate patch whose
    # approx score is within EPS of the max. Correctness needs
    # EPS >= 2*max|err|; measured err is +-7 (fp8e4), so 30 is ~2x margin.
    EPS = 30.0
    amax = float(min_d2.max())
    S = np.flatnonzero(min_d2 >= amax - EPS)
    if len(S) > 2048:
        S = np.argsort(min_d2)[-2048:]
    cross_S = p64[S] @ b64.T
    d2_S = p_sq[S, None] + m_sq[None, :] - 2.0 * cross_S
    score = np.sqrt(max(float(np.maximum(d2_S, 0.0).min(axis=1).max()), 0.0))
    return np.asarray(score, dtype=np.float32)



# revision 3
# speedup vs baseline: 1.3287x; 1.3287x over previous
"""PatchCore anomaly score kernel for 8 trn2 NeuronCores.

score = sqrt(max_n min_m ||patches[n] - memory_bank[m]||^2)

Device (per core, memory_bank sharded 4096 rows/core):
  acc[p, n] = max_mt (2*patches[n].bank[mt*128+p] + C - m_sq[mt*128+p])
Host:
  r[n] = max_c max_p acc_c[p, n]; min_d2[n] = p_sq[n] + C - r[n]
  score = sqrt(max_n min_d2), refined exactly for near-max candidates.

Pipeline (measured per 512-col unit): PE fp8 DoubleRow matmul 181ns,
ACT pair-evac [128,1024] from 2 PSUM banks 559ns (bias=C-m_sq, scale=2),
DVE quad tensor_max [128,2048] bf16 1149ns. ACT 4472 vs DVE 4596 per
m-tile of 16 units -> ~290ns/unit steady, ~147us/core projected.
"""

import sys

import numpy as np

try:
    import concourse.bass as bass
except ImportError:
    sys.path.insert(0, "/opt/trn_rl_repo")
    import concourse.bass as bass

import concourse.bacc as bacc
import concourse.tile as tile
from concourse import mybir
from concourse.bass_utils import run_bass_kernel_spmd

import ml_dtypes

N = 8192          # patches
M_TOTAL = 32768   # memory bank rows
D = 512           # feature dim
N_CORES = 8
M = M_TOTAL // N_CORES   # 4096 bank rows per core

KP = 4            # k-chunks of 128 over D
NT = N // 512     # 16 n-tiles of 512 patches
MT = M // 128     # 32 m-tiles of 128 bank rows
NG = 4            # n-groups (NT // NG tiles each, outer loop)
GT = NT // NG     # 4 n-tiles per group


def _build_nc(repeat=1):
    nc = bacc.Bacc(None, target_bir_lowering=False)
    f32 = mybir.dt.float32
    bf16 = mybir.dt.bfloat16
    fp8 = mybir.dt.float8e4

    # dim1 index ci = d//128; slice [:, 2c:2c+2, :] = K rows [c*256,(c+1)*256)
    at_d = nc.dram_tensor("at", [128, KP, N], fp8, kind="ExternalInput")
    bt_d = nc.dram_tensor("bt", [128, KP, M], fp8, kind="ExternalInput")
    msq_d = nc.dram_tensor("msq", [128, MT], f32, kind="ExternalInput")
    # out[p, n] = max over m-tiles of 2*a.b + (C - m_sq); host maxes over p
    out_d = nc.dram_tensor("out", [128, N], bf16, kind="ExternalOutput")

    with tile.TileContext(nc) as tc:
        with (
            tc.tile_pool(name="at", bufs=1) as at_pool,
            tc.tile_pool(name="bt", bufs=1) as bt_pool,
            tc.tile_pool(name="msq", bufs=1) as msq_pool,
            tc.tile_pool(name="acc", bufs=1) as acc_pool,
            tc.tile_pool(name="ev", bufs=4) as ev_pool,
            tc.tile_pool(name="psum", bufs=4, space="PSUM") as psum_pool,
        ):
            msq_t = msq_pool.tile([128, MT], f32)
            nc.gpsimd.dma_start(msq_t[:], msq_d[:])
            acc_t = acc_pool.tile([128, N], bf16)

            # patches first, in group order (group 0 unblocks compute),
            # split small so DMA queues round-robin across engines
            at_t = at_pool.tile([128, KP, N], fp8)
            for g in range(NG):
                for ci in range(KP):
                    nc.gpsimd.dma_start(
                        at_t[:, ci, bass.ts(g, N // NG)],
                        at_d[:, ci, bass.ts(g, N // NG)],
                    )
            bt_t = bt_pool.tile([128, KP, M], fp8)
            for j in range(8):
                for ci in range(KP):
                    nc.gpsimd.dma_start(
                        bt_t[:, ci, bass.ts(j, M // 8)],
                        bt_d[:, ci, bass.ts(j, M // 8)],
                    )

            def compute_body():
                for g in range(NG):
                    for m in range(MT):
                        evq = None
                        for pj in range(GT // 2):  # pairs of n-tiles
                            nt0 = g * GT + 2 * pj
                            ps = psum_pool.tile([128, 1024], f32)
                            for h in range(2):
                                for c in range(2):
                                    nc.tensor.matmul(
                                        ps[:, bass.ts(h, 512)],
                                        bt_t[:, 2 * c : 2 * c + 2,
                                             bass.ts(m, 128)],
                                        at_t[:, 2 * c : 2 * c + 2,
                                             bass.ts(nt0 + h, 512)],
                                        start=(c == 0),
                                        stop=(c == 1),
                                        perf_mode=mybir.MatmulPerfMode.DoubleRow,
                                    )
                            if m == 0:
                                nc.scalar.activation(
                                    acc_t[:, nt0 * 512 : (nt0 + 2) * 512],
                                    ps[:],
                                    mybir.ActivationFunctionType.Identity,
                                    bias=msq_t[:, m : m + 1], scale=2.0,
                                )
                                continue
                            # two pair-evacs fill one [128,2048] quad buffer,
                            # merged by a single DVE max into the accumulator
                            if pj % 2 == 0:
                                evq = ev_pool.tile([128, 2048], bf16)
                            nc.scalar.activation(
                                evq[:, bass.ts(pj % 2, 1024)], ps[:],
                                mybir.ActivationFunctionType.Identity,
                                bias=msq_t[:, m : m + 1], scale=2.0,
                            )
                            if pj % 2 == 1:
                                q0 = g * GT + 2 * (pj - 1)
                                a = acc_t[:, q0 * 512 : (q0 + 4) * 512]
                                nc.vector.tensor_max(a, a, evq[:])
                    # flush this group's accumulator columns
                    for j in range(4):
                        col = g * GT * 512 + j * 512
                        nc.gpsimd.dma_start(
                            out_d[:, col : col + 512],
                            acc_t[:, col : col + 512],
                        )

            if repeat == 1:
                compute_body()
            else:
                with tc.For_i(0, repeat):
                    compute_body()

    nc.finalize()
    return nc


_NC = None


def prepare_in_maps(patches: np.ndarray, memory_bank: np.ndarray):
    m_sq = np.sum(memory_bank.astype(np.float64) ** 2, axis=1)
    C = float(np.mean(m_sq))
    at_np = np.ascontiguousarray(
        patches.T.astype(ml_dtypes.float8_e4m3)
        .reshape(KP, 128, N).transpose(1, 0, 2)
    )
    in_maps = []
    for c in range(N_CORES):
        bank_c = memory_bank[c * M : (c + 1) * M]
        bt_np = np.ascontiguousarray(
            bank_c.T.astype(ml_dtypes.float8_e4m3)
            .reshape(KP, 128, M).transpose(1, 0, 2)
        )
        msq_c = C - m_sq[c * M : (c + 1) * M]
        msq_np = np.ascontiguousarray(
            msq_c.reshape(MT, 128).T
        ).astype(np.float32)
        in_maps.append({"at": at_np, "bt": bt_np, "msq": msq_np})
    return in_maps


def kernel(patches: np.ndarray, memory_bank: np.ndarray) -> np.ndarray:
    global _NC
    if _NC is None:
        _NC = _build_nc()
    nc = _NC

    p64 = patches.astype(np.float64)
    b64 = memory_bank.astype(np.float64)
    p_sq = np.sum(p64 * p64, axis=1)          # [N]
    m_sq = np.sum(b64 * b64, axis=1)          # [M_TOTAL]
    C = float(np.mean(m_sq))

    in_maps = prepare_in_maps(patches, memory_bank)

    br = run_bass_kernel_spmd(nc, in_maps, list(range(N_CORES)))
    r = np.max(
        np.stack(
            [np.asarray(br.results[c]["out"], np.float64).max(axis=0)
             for c in range(N_CORES)]
        ),
        axis=0,
    )
    min_d2 = np.maximum(p_sq + C - r, 0.0)

    # Host refinement: device min_d2 is approximate (fp8 matmul + bf16 max
    # accumulation). Recompute exact d2 rows for every candidate patch whose
    # approx score is within EPS of the max. Correctness needs
    # EPS >= 2*max|err|; measured err is +-7 (fp8e4), so 30 is ~2x margin.
    EPS = 30.0
    amax = float(min_d2.max())
    S = np.flatnonzero(min_d2 >= amax - EPS)
    if len(S) > 2048:
        S = np.argsort(min_d2)[-2048:]
    cross_S = p64[S] @ b64.T
    d2_S = p_sq[S, None] + m_sq[None, :] - 2.0 * cross_S
    score = np.sqrt(max(float(np.maximum(d2_S, 0.0).min(axis=1).max()), 0.0))
    return np.asarray(score, dtype=np.float32)


# revision 7
# speedup vs baseline: 1.4018x; 1.0550x over previous
"""PatchCore anomaly score kernel for 8 trn2 NeuronCores.

score = sqrt(max_n min_m ||patches[n] - memory_bank[m]||^2)

Device (per core, memory_bank sharded 4096 rows/core):
  acc[p, n] = max_mt (2*patches[n].bank[mt*128+p] + C - m_sq[mt*128+p])
Host:
  r[n] = max_c max_p acc_c[p, n]; min_d2[n] = p_sq[n] + C - r[n]
  score = sqrt(max_n min_d2), refined exactly for near-max candidates.

Pipeline (measured per 512-col unit): PE fp8 DoubleRow matmul 181ns,
ACT pair-evac [128,1024] from 2 PSUM banks 559ns (bias=C-m_sq, scale=2),
DVE quad tensor_max [128,2048] bf16 1149ns. ACT 4472 vs DVE 4596 per
m-tile of 16 units -> ~290ns/unit steady, ~147us/core projected.
"""

import sys

import numpy as np

try:
    import concourse.bass as bass
except ImportError:
    sys.path.insert(0, "/opt/trn_rl_repo")
    import concourse.bass as bass

import concourse.bacc as bacc
import concourse.tile as tile
from concourse import mybir
from concourse.bass_utils import run_bass_kernel_spmd

import ml_dtypes

N = 8192          # patches
M_TOTAL = 32768   # memory bank rows
D = 512           # feature dim
N_CORES = 8
M = M_TOTAL // N_CORES   # 4096 bank rows per core

KP = 4            # k-chunks of 128 over D
NT = N // 512     # 16 n-tiles of 512 patches
MT = M // 128     # 32 m-tiles of 128 bank rows
NG = 4            # n-groups (NT // NG tiles each, outer loop)
GT = NT // NG     # 4 n-tiles per group


def _build_nc(repeat=1, mode="full"):
    # mode: debug/timing decomposition
    #  "full"   - real kernel
    #  "nodve"  - ACT evacs to rotating ev only, no acc merge (wrong output)
    #  "noact"  - matmuls + flush only (wrong output)
    #  "dumpdve"- DVE max into rotating dump instead of acc RMW (wrong output)
    nc = bacc.Bacc(None, target_bir_lowering=False)
    f32 = mybir.dt.float32
    bf16 = mybir.dt.bfloat16
    fp8 = mybir.dt.float8e4

    # dim1 index ci = d//128; slice [:, 2c:2c+2, :] = K rows [c*256,(c+1)*256)
    at_d = nc.dram_tensor("at", [128, KP, N], fp8, kind="ExternalInput")
    bt_d = nc.dram_tensor("bt", [128, KP, M], fp8, kind="ExternalInput")
    msq_d = nc.dram_tensor("msq", [128, MT], f32, kind="ExternalInput")
    # out[p, n] = max over m-tiles of 2*a.b + (C - m_sq); host maxes over p
    out_d = nc.dram_tensor("out", [128, N], bf16, kind="ExternalOutput")

    with tile.TileContext(nc) as tc:
        with (
            tc.tile_pool(name="at", bufs=1) as at_pool,
            tc.tile_pool(name="bt", bufs=1) as bt_pool,
            tc.tile_pool(name="msq", bufs=1) as msq_pool,
            tc.tile_pool(name="acc", bufs=1) as acc_pool,
            tc.tile_pool(name="ev", bufs=4) as ev_pool,
            tc.tile_pool(name="psum", bufs=2, space="PSUM") as psum_pool,
        ):
            msq_t = msq_pool.tile([128, MT], f32)
            nc.gpsimd.dma_start(msq_t[:], msq_d[:])
            acc_t = acc_pool.tile([128, N], bf16)

            # patches first, in group order (group 0 unblocks compute),
            # split small so DMA queues round-robin across engines
            at_t = at_pool.tile([128, KP, N], fp8)
            for g in range(NG):
                for ci in range(KP):
                    nc.gpsimd.dma_start(
                        at_t[:, ci, bass.ts(g, N // NG)],
                        at_d[:, ci, bass.ts(g, N // NG)],
                    )
            bt_t = bt_pool.tile([128, KP, M], fp8)
            for j in range(8):
                for ci in range(KP):
                    nc.gpsimd.dma_start(
                        bt_t[:, ci, bass.ts(j, M // 8)],
                        bt_d[:, ci, bass.ts(j, M // 8)],
                    )

            def compute_body():
                for g in range(NG):
                    for m in range(MT):
                        # one 4-bank PSUM quad per m-tile: 8 matmuls in,
                        # ONE quad ACT evac out (multi-bank reads of fully
                        # written banks measured ~4x faster than pairs)
                        nt0 = g * GT
                        ps = psum_pool.tile([128, 2048], f32)
                        for h in range(GT):
                            for c in range(2):
                                nc.tensor.matmul(
                                    ps[:, bass.ts(h, 512)],
                                    bt_t[:, 2 * c : 2 * c + 2,
                                         bass.ts(m, 128)],
                                    at_t[:, 2 * c : 2 * c + 2,
                                         bass.ts(nt0 + h, 512)],
                                    start=(c == 0),
                                    stop=(c == 1),
                                    perf_mode=mybir.MatmulPerfMode.DoubleRow,
                                )
                        if mode == "noact":
                            continue
                        if m == 0:
                            nc.scalar.activation(
                                acc_t[:, nt0 * 512 : (nt0 + GT) * 512],
                                ps[:],
                                mybir.ActivationFunctionType.Identity,
                                bias=msq_t[:, m : m + 1], scale=2.0,
                            )
                            continue
                        evq = ev_pool.tile([128, 2048], bf16)
                        nc.scalar.activation(
                            evq[:], ps[:],
                            mybir.ActivationFunctionType.Identity,
                            bias=msq_t[:, m : m + 1], scale=2.0,
                        )
                        if mode == "nodve":
                            continue
                        a = acc_t[:, nt0 * 512 : (nt0 + GT) * 512]
                        if mode == "dumpdve":
                            dmp = ev_pool.tile([128, 2048], bf16, name="dmp")
                            nc.vector.tensor_max(dmp[:], a, evq[:])
                        else:
                            nc.vector.tensor_max(a, a, evq[:])
                    # flush this group's accumulator columns
                    for j in range(4):
                        col = g * GT * 512 + j * 512
                        nc.gpsimd.dma_start(
                            out_d[:, col : col + 512],
                            acc_t[:, col : col + 512],
                        )

            if repeat == 1:
                compute_body()
            else:
                with tc.For_i(0, repeat):
                    compute_body()

    nc.finalize()
    return nc


_NC = None


def prepare_in_maps(patches: np.ndarray, memory_bank: np.ndarray):
    m_sq = np.sum(memory_bank.astype(np.float64) ** 2, axis=1)
    C = float(np.mean(m_sq))
    at_np = np.ascontiguousarray(
        patches.T.astype(ml_dtypes.float8_e4m3)
        .reshape(KP, 128, N).transpose(1, 0, 2)
    )
    in_maps = []
    for c in range(N_CORES):
        bank_c = memory_bank[c * M : (c + 1) * M]
        bt_np = np.ascontiguousarray(
            bank_c.T.astype(ml_dtypes.float8_e4m3)
            .reshape(KP, 128, M).transpose(1, 0, 2)
        )
        msq_c = C - m_sq[c * M : (c + 1) * M]
        msq_np = np.ascontiguousarray(
            msq_c.reshape(MT, 128).T
        ).astype(np.float32)
        in_maps.append({"at": at_np, "bt": bt_np, "msq": msq_np})
    return in_maps


def kernel(patches: np.ndarray, memory_bank: np.ndarray) -> np.ndarray:
    global _NC
    if _NC is None:
        _NC = _build_nc()
    nc = _NC

    p64 = patches.astype(np.float64)
    b64 = memory_bank.astype(np.float64)
    p_sq = np.sum(p64 * p64, axis=1)          # [N]
    m_sq = np.sum(b64 * b64, axis=1)          # [M_TOTAL]
    C = float(np.mean(m_sq))

    in_maps = prepare_in_maps(patches, memory_bank)

    br = run_bass_kernel_spmd(nc, in_maps, list(range(N_CORES)))
    r = np.max(
        np.stack(
            [np.asarray(br.results[c]["out"], np.float64).max(axis=0)
             for c in range(N_CORES)]
        ),
        axis=0,
    )
    min_d2 = np.maximum(p_sq + C - r, 0.0)

    # Host refinement: device min_d2 is approximate (fp8 matmul + bf16 max
    # accumulation). Recompute exact d2 rows for every candidate patch whose
    # approx score is within EPS of the max. Correctness needs
    # EPS >= 2*max|err|; measured err is +-7 (fp8e4), so 30 is ~2x margin.
    EPS = 30.0
    amax = float(min_d2.max())
    S = np.flatnonzero(min_d2 >= amax - EPS)
    if len(S) > 2048:
        S = np.argsort(min_d2)[-2048:]
    cross_S = p64[S] @ b64.T
    d2_S = p_sq[S, None] + m_sq[None, :] - 2.0 * cross_S
    score = np.sqrt(max(float(np.maximum(d2_S, 0.0).min(axis=1).max()), 0.0))
    return np.asarray(score, dtype=np.float32)


# revision 9
# speedup vs baseline: 1.4687x; 1.0477x over previous
"""PatchCore anomaly score kernel for 8 trn2 NeuronCores.

score = sqrt(max_n min_m ||patches[n] - memory_bank[m]||^2)

Device (per core, memory_bank sharded 4096 rows/core):
  acc[p, n] = max_mt (2*patches[n].bank[mt*128+p] + C - m_sq[mt*128+p])
Host:
  r[n] = max_c max_p acc_c[p, n]; min_d2[n] = p_sq[n] + C - r[n]
  score = sqrt(max_n min_d2), refined exactly for near-max candidates.

Pipeline: per m-tile, 2 n-quads ([128,2048] PSUM, 4 banks each). Measured
unit costs (512 cols): PE 2xDR matmul 181ns, ACT quad evac ~480ns, DVE quad
merge ~287ns, DVE quad STT (bias+max straight from PSUM) ~564ns. Alternating
quad mechanisms 3xACT : 1xSTT per 2 m-steps balances ACT (~360ns/u) against
DVE (~356ns/u) under the 8-bank PSUM cap -> ~190us/core target.
"""

import sys

import numpy as np

try:
    import concourse.bass as bass
except ImportError:
    sys.path.insert(0, "/opt/trn_rl_repo")
    import concourse.bass as bass

import concourse.bacc as bacc
import concourse.tile as tile
from concourse import mybir
from concourse.bass_utils import run_bass_kernel_spmd

import ml_dtypes

N = 8192          # patches
M_TOTAL = 32768   # memory bank rows
D = 512           # feature dim
N_CORES = 8
M = M_TOTAL // N_CORES   # 4096 bank rows per core

KP = 4            # k-chunks of 128 over D
NT = N // 512     # 16 n-tiles of 512 patches
MT = M // 128     # 32 m-tiles of 128 bank rows
NG = 2            # n-groups (outer loop)
GT = NT // NG     # 8 n-tiles per group = 2 quads per m-step


def _build_nc(repeat=1, mode="full"):
    nc = bacc.Bacc(None, target_bir_lowering=False)
    f32 = mybir.dt.float32
    bf16 = mybir.dt.bfloat16
    fp8 = mybir.dt.float8e4

    # dim1 index ci = d//128; slice [:, 2c:2c+2, :] = K rows [c*256,(c+1)*256)
    at_d = nc.dram_tensor("at", [128, KP, N], fp8, kind="ExternalInput")
    bt_d = nc.dram_tensor("bt", [128, KP, M], fp8, kind="ExternalInput")
    msq_d = nc.dram_tensor("msq", [128, MT], f32, kind="ExternalInput")
    # out[p, n] = max over m-tiles of 2*a.b + (C - m_sq); host maxes over p
    out_d = nc.dram_tensor("out", [128, N], bf16, kind="ExternalOutput")

    with tile.TileContext(nc) as tc:
        with (
            tc.tile_pool(name="at", bufs=1) as at_pool,
            tc.tile_pool(name="bt", bufs=1) as bt_pool,
            tc.tile_pool(name="msq", bufs=1) as msq_pool,
            tc.tile_pool(name="acc", bufs=1) as acc_pool,
            tc.tile_pool(name="ev", bufs=4) as ev_pool,
            tc.tile_pool(name="psum", bufs=2, space="PSUM") as psum_pool,
        ):
            msq_t = msq_pool.tile([128, MT], f32)
            nc.gpsimd.dma_start(msq_t[:], msq_d[:])
            acc_t = acc_pool.tile([128, N], bf16)

            # load order: group-0 patches + first bank chunks first so the
            # m-loop starts after ~2.5MB, not after the full 6MB
            at_t = at_pool.tile([128, KP, N], fp8)
            bt_t = bt_pool.tile([128, KP, M], fp8)

            def load_at(g):
                for ci in range(KP):
                    nc.gpsimd.dma_start(
                        at_t[:, ci, bass.ts(g, N // NG)],
                        at_d[:, ci, bass.ts(g, N // NG)],
                    )

            def load_bt(j):
                for ci in range(KP):
                    nc.gpsimd.dma_start(
                        bt_t[:, ci, bass.ts(j, M // 8)],
                        bt_d[:, ci, bass.ts(j, M // 8)],
                    )

            load_at(0)
            for j in range(4):
                load_bt(j)
            load_at(1)
            for j in range(4, 8):
                load_bt(j)

            def fill_quad(ps, m, nt0):
                for h in range(4):
                    for c in range(2):
                        nc.tensor.matmul(
                            ps[:, bass.ts(h, 512)],
                            bt_t[:, 2 * c : 2 * c + 2, bass.ts(m, 128)],
                            at_t[:, 2 * c : 2 * c + 2,
                                 bass.ts(nt0 + h, 512)],
                            start=(c == 0),
                            stop=(c == 1),
                            perf_mode=mybir.MatmulPerfMode.DoubleRow,
                        )

            def compute_body():
                for g in range(NG):
                    for m in range(MT):
                        for q in range(GT // 4):  # 2 quads per m-step
                            nt0 = g * GT + 4 * q
                            a = acc_t[:, nt0 * 512 : (nt0 + 4) * 512]
                            ps = psum_pool.tile([128, 2048], f32)
                            fill_quad(ps, m, nt0)
                            if m == 0:
                                # init accumulator straight from ACT
                                nc.scalar.activation(
                                    a, ps[:],
                                    mybir.ActivationFunctionType.Identity,
                                    bias=msq_t[:, m : m + 1], scale=2.0,
                                )
                            elif m % 2 == 1 and q == 1:
                                # DVE-direct quad: bias+max straight from
                                # PSUM in one scalar_tensor_tensor
                                nc.vector.scalar_tensor_tensor(
                                    a, ps[:], msq_t[:, m : m + 1], a,
                                    mybir.AluOpType.add, mybir.AluOpType.max,
                                )
                            else:
                                ev = ev_pool.tile([128, 2048], bf16)
                                nc.scalar.activation(
                                    ev[:], ps[:],
                                    mybir.ActivationFunctionType.Identity,
                                    bias=msq_t[:, m : m + 1], scale=2.0,
                                )
                                nc.vector.tensor_max(a, a, ev[:])
                    # flush this group's accumulator columns
                    for j in range(GT):
                        col = (g * GT + j) * 512
                        nc.gpsimd.dma_start(
                            out_d[:, col : col + 512],
                            acc_t[:, col : col + 512],
                        )

            if repeat == 1:
                compute_body()
            else:
                with tc.For_i(0, repeat):
                    compute_body()

    nc.finalize()
    return nc


_NC = None


def prepare_in_maps(patches: np.ndarray, memory_bank: np.ndarray):
    m_sq = np.sum(memory_bank.astype(np.float64) ** 2, axis=1)
    C = float(np.mean(m_sq))
    at_np = np.ascontiguousarray(
        patches.T.astype(ml_dtypes.float8_e4m3)
        .reshape(KP, 128, N).transpose(1, 0, 2)
    )
    in_maps = []
    for c in range(N_CORES):
        bank_c = memory_bank[c * M : (c + 1) * M]
        bt_np = np.ascontiguousarray(
            bank_c.T.astype(ml_dtypes.float8_e4m3)
            .reshape(KP, 128, M).transpose(1, 0, 2)
        )
        msq_c = C - m_sq[c * M : (c + 1) * M]
        msq_np = np.ascontiguousarray(
            msq_c.reshape(MT, 128).T
        ).astype(np.float32)
        in_maps.append({"at": at_np, "bt": bt_np, "msq": msq_np})
    return in_maps


def kernel(patches: np.ndarray, memory_bank: np.ndarray) -> np.ndarray:
    global _NC
    if _NC is None:
        _NC = _build_nc()
    nc = _NC

    p64 = patches.astype(np.float64)
    b64 = memory_bank.astype(np.float64)
    p_sq = np.sum(p64 * p64, axis=1)          # [N]
    m_sq = np.sum(b64 * b64, axis=1)          # [M_TOTAL]
    C = float(np.mean(m_sq))

    in_maps = prepare_in_maps(patches, memory_bank)

    br = run_bass_kernel_spmd(nc, in_maps, list(range(N_CORES)))
    r = np.max(
        np.stack(
            [np.asarray(br.results[c]["out"], np.float64).max(axis=0)
             for c in range(N_CORES)]
        ),
        axis=0,
    )
    min_d2 = np.maximum(p_sq + C - r, 0.0)

    # Host refinement: device min_d2 is approximate (fp8 matmul + bf16 max
    # accumulation). Recompute exact d2 rows for every candidate patch whose
    # approx score is within EPS of the max. Correctness needs
    # EPS >= 2*max|err|; measured err is +-7 (fp8e4), so 30 is ~2x margin.
    EPS = 30.0
    amax = float(min_d2.max())
    S = np.flatnonzero(min_d2 >= amax - EPS)
    if len(S) > 2048:
        S = np.argsort(min_d2)[-2048:]
    cross_S = p64[S] @ b64.T
    d2_S = p_sq[S, None] + m_sq[None, :] - 2.0 * cross_S
    score = np.sqrt(max(float(np.maximum(d2_S, 0.0).min(axis=1).max()), 0.0))
    return np.asarray(score, dtype=np.float32)
